# revision 24
# baseline (speedup 1.0000x reference)
"""Trainium2 Bass kernel for nn_ADCLayer (GAT-style message passing).

Math (reference reduction):
  sj = X @ (Wv @ aw[:d]) + bv.aw[:d]          (per-column score, j axis)
  si = X @ (Wv @ aw[d:]) + bv.aw[d:] + ab     (per-row score, i axis)
  alpha = A * exp(leaky_relu(si[i] + sj[j]))  (unnormalized transition)
  T = alpha / rowsum(alpha)
  H = X@Wk0 + (T X)@Wk1 + (T^2 X)@Wk2 + sum_k bk[k]   (last ref hop is dead code)
  out = relu(H)

Device algebra (per core, partition=j layout, zero big transposes, both
hops run on RAW alphaT so nothing waits for normalization):
  alphaT[j, i] = A^T[j, i] * exp(lrelu(si[i] + sj[j]))      (bf16)
  r via ones-stationary matmul -> (1, I); recip_col via 8 tiny PE
  transposes + exact reciprocal; r128 row-broadcast via ones outer-product.
  P2 = X@Wk2 ; G2 = recip_col x (alphaT^T P2) + bks -> pairwise AllGather.
  P1 = X@Wk1 ; S-own = P1 + G2-own ; S-other = P1 + masked partner half.
  H_psum = (r x X)@Wk0 + alphaT^T S ; out = relu(recip_col * H_psum).

Sharding: 8 cores = 4 batches x 2 row-halves; j axis permuted per core
(own half first) so own j-tiles have uniform local indices.

DMA strategy: few big multi-dim DMAs (issue op costs ~0.65us of engine
time each), priority emission order, all on the sync queue.
"""

import numpy as np

B, N, DIN, DOUT = 4, 2048, 512, 512
HALF = N // 2          # rows per core
NCORES = 8
JT = N // 128          # 16 j tiles
IT = HALF // 128       # 8 i tiles (also own j tiles)
DT = DIN // 128        # 4 d tiles

_CACHE = {}


def _build():
    import concourse.bacc as bacc
    import concourse.tile as tile
    import concourse.mybir as mybir
    from concourse.bass import ts

    f32 = mybir.dt.float32
    bf16 = mybir.dt.bfloat16
    AOP = mybir.AluOpType
    AF = mybir.ActivationFunctionType

    nc = bacc.Bacc("TRN2", target_bir_lowering=False, debug=False,
                   num_devices=NCORES)

    AT = nc.declare_dram_parameter("AT", [N, HALF], bf16, isOutput=False)
    XT = nc.declare_dram_parameter("XT", [DIN, N], bf16, isOutput=False)
    WK0 = nc.declare_dram_parameter("WK0", [DIN, DOUT], bf16, isOutput=False)
    WK1 = nc.declare_dram_parameter("WK1", [DIN, DOUT], bf16, isOutput=False)
    WK2 = nc.declare_dram_parameter("WK2", [DIN, DOUT], bf16, isOutput=False)
    SJT = nc.declare_dram_parameter("SJT", [128, JT], f32, isOutput=False)
    SIB = nc.declare_dram_parameter("SIB", [128, HALF], f32, isOutput=False)
    BKS = nc.declare_dram_parameter("BKS", [128, DOUT], f32, isOutput=False)
    MLO = nc.declare_dram_parameter("MLO", [1, 1], f32, isOutput=False)
    MHI = nc.declare_dram_parameter("MHI", [1, 1], f32, isOutput=False)
    OUT = nc.declare_dram_parameter("out", [HALF, DOUT], f32, isOutput=True)

    g2_in = nc.dram_tensor("g2_in", [HALF, DOUT], bf16)
    g2_all = nc.dram_tensor("g2_all", [N, DOUT], bf16)

    with tile.TileContext(nc) as tc:
        with tc.tile_pool(name="sb", bufs=1) as sb:
            # ---- input DMAs: few, big, priority-ordered, sync queue ----
            at3 = AT[:, :].rearrange("(q p) i -> p q i", p=128)  # (128,16,1024)
            sjt = sb.tile([128, JT], f32, tag="sjt", bufs=1)
            nc.sync.dma_start(out=sjt[:], in_=SJT[:, :])
            sib = sb.tile([128, HALF], f32, tag="sib", bufs=1)
            nc.sync.dma_start(out=sib[:], in_=SIB[:, :])
            atp = []
            for q in range(IT):
                t = sb.tile([128, 2 * HALF], bf16, tag=f"atp{q}", bufs=1)
                atp.append(t)

            def at_dma(q):
                nc.sync.dma_start(
                    out=atp[q][:].rearrange("p (h i) -> p h i", i=HALF),
                    in_=at3[:, 2 * q:2 * q + 2, :])

            at_dma(0)
            xt = []
            for d in range(DT):
                t = sb.tile([128, N], bf16, tag=f"xt{d}", bufs=1)
                nc.sync.dma_start(out=t[:], in_=XT[ts(d, 128), :])
                xt.append(t)
            wk = {}
            for nm, src in (("wk2", WK2),):
                t = sb.tile([128, DT * DOUT], bf16, tag=nm, bufs=1)
                nc.sync.dma_start(
                    out=t[:].rearrange("p (d n) -> p d n", n=DOUT),
                    in_=src[:, :].rearrange("(d p) n -> p d n", p=128))
                wk[nm] = t
            for q in range(1, IT):
                at_dma(q)
            for nm, src in (("wk1", WK1), ("wk0", WK0)):
                t = sb.tile([128, DT * DOUT], bf16, tag=nm, bufs=1)
                nc.sync.dma_start(
                    out=t[:].rearrange("p (d n) -> p d n", n=DOUT),
                    in_=src[:, :].rearrange("(d p) n -> p d n", p=128))
                wk[nm] = t
            bks = sb.tile([128, DOUT], f32, tag="bks", bufs=1)
            nc.sync.dma_start(out=bks[:], in_=BKS[:, :])
            mlo = sb.tile([128, 1], f32, tag="mlo", bufs=1)
            nc.sync.dma_start(out=mlo[:], in_=MLO[:, :].to_broadcast((128, 1)))
            mhi = sb.tile([128, 1], f32, tag="mhi", bufs=1)
            nc.sync.dma_start(out=mhi[:], in_=MHI[:, :].to_broadcast((128, 1)))
            ones = sb.tile([128, 1], bf16, tag="ones", bufs=1)
            nc.vector.memset(ones[:], 1.0)
            ones1f = sb.tile([1, 1], f32, tag="ones1f", bufs=1)
            nc.vector.memset(ones1f[:], 1.0)
            ones128f = sb.tile([1, 128], f32, tag="ones128f", bufs=1)
            nc.vector.memset(ones128f[:], 1.0)

            def atS(jt, off, size):
                q, h = divmod(jt, 2)
                return atp[q][:, h * HALF + off: h * HALF + off + size]

            p1 = [None] * JT
            with tc.tile_pool(name="psAll", bufs=1, space="PSUM") as psA:
                # ---- phase 1: elementwise alphaT (raw) + r + P2 --------
                r_ps = psA.tile([1, HALF], f32, tag="r", bufs=1)
                for jt in range(JT):
                    z = sb.tile([128, HALF], f32, tag="z", bufs=3)
                    nc.scalar.activation(z[:], sib[:], AF.Identity,
                                         bias=sjt[:, jt:jt + 1], scale=1.0)
                    nc.vector.scalar_tensor_tensor(z[:], z[:], 0.2, z[:],
                                                   op0=AOP.mult, op1=AOP.max)
                    e = sb.tile([128, HALF], bf16, tag="e", bufs=3)
                    nc.scalar.activation(e[:], z[:], AF.Exp)
                    nc.vector.tensor_mul(atS(jt, 0, HALF), atS(jt, 0, HALF),
                                         e[:])
                    for h in range(2):
                        nc.tensor.matmul(r_ps[:, ts(h, 512)], lhsT=ones[:],
                                         rhs=atS(jt, h * 512, 512),
                                         start=(jt == 0), stop=(jt == JT - 1))

                p2 = []
                for jt in range(JT):
                    pp2 = psA.tile([128, DOUT], f32, tag="mm", bufs=2)
                    for d in range(DT):
                        nc.tensor.matmul(pp2[:], lhsT=xt[d][:, ts(jt, 128)],
                                         rhs=wk["wk2"][:, ts(d, DOUT)],
                                         start=(d == 0), stop=(d == DT - 1))
                    t2 = sb.tile([128, DOUT], bf16, tag=f"p2_{jt}", bufs=1)
                    nc.vector.tensor_copy(t2[:], pp2[:])
                    p2.append(t2)

                # ---- hop A on RAW alphaT: two 4-bank passes ------------
                g2big = sb.tile([128, IT * DOUT], bf16, tag="g2big", bufs=1)
                ua = [None] * IT
                for half in range(2):
                    for i in range(4):
                        it = half * 4 + i
                        ua[it] = psA.tile([128, DOUT], f32, tag=f"ua{i}",
                                          bufs=1, name=f"ua{half}_{i}")
                    for jt in range(JT):
                        for i in range(4):
                            it = half * 4 + i
                            nc.tensor.matmul(ua[it][:],
                                             lhsT=atS(jt, it * 128, 128),
                                             rhs=p2[jt][:],
                                             start=(jt == 0),
                                             stop=(jt == JT - 1))

                    if half == 0:
                        # r -> recip_col (8 tiny PE transposes + recip)
                        r_sb = sb.tile([1, HALF], f32, tag="rsb", bufs=1)
                        nc.vector.tensor_copy(r_sb[:], r_ps[:])
                        rt = psA.tile([128, IT], f32, tag="r", bufs=1,
                                      name="rt")
                        for c in range(IT):
                            nc.tensor.matmul(rt[:, c:c + 1],
                                             lhsT=r_sb[0:1, ts(c, 128)],
                                             rhs=ones1f[:],
                                             is_transpose=True,
                                             start=True, stop=True)
                        rr_col = sb.tile([128, IT], f32, tag="rrc", bufs=1)
                        nc.vector.reciprocal(rr_col[:], rt[:])

                    # G2 = recip_col * UA + bks into the packed g2 tile
                    for i in range(4):
                        it = half * 4 + i
                        nc.vector.scalar_tensor_tensor(
                            g2big[:, ts(it, DOUT)], ua[it][:],
                            rr_col[:, it:it + 1], bks[:],
                            op0=AOP.mult, op1=AOP.add)

                nc.sync.dma_start(
                    out=g2_in[:, :].rearrange("(t p) n -> p t n", p=128),
                    in_=g2big[:].rearrange("p (t n) -> p t n", n=DOUT))

                nc.gpsimd.collective_compute(
                    "AllGather", AOP.bypass,
                    ins=[g2_in.ap().opt()],
                    outs=[g2_all.ap().opt()],
                    replica_groups=[[0, 1], [2, 3], [4, 5], [6, 7]],
                )

                # r128 row-broadcast via ones outer product (into psum)
                r128 = psA.tile([128, HALF], f32, tag="r", bufs=1,
                                name="r128")
                for h in range(2):
                    nc.tensor.matmul(r128[:, ts(h, 512)], lhsT=ones128f[:],
                                     rhs=r_sb[0:1, ts(h, 512)],
                                     start=True, stop=True)
                xts = []
                for d in range(DT):
                    t = sb.tile([128, HALF], bf16, tag=f"xts{d}", bufs=1)
                    nc.vector.tensor_mul(t[:], xt[d][:, 0:HALF], r128[:])
                    xts.append(t)

                # ---- cover window: P1 (+S-own fused) -------------------
                for jt in range(JT):
                    pp1 = psA.tile([128, DOUT], f32, tag="mm", bufs=2)
                    for d in range(DT):
                        nc.tensor.matmul(pp1[:], lhsT=xt[d][:, ts(jt, 128)],
                                         rhs=wk["wk1"][:, ts(d, DOUT)],
                                         start=(d == 0), stop=(d == DT - 1))
                    t1 = sb.tile([128, DOUT], bf16, tag=f"p1_{jt}", bufs=1)
                    if jt < IT:
                        nc.vector.scalar_tensor_tensor(
                            t1[:], pp1[:], 1.0, g2big[:, ts(jt, DOUT)],
                            op0=AOP.mult, op1=AOP.add)
                    else:
                        nc.vector.tensor_copy(t1[:], pp1[:])
                    p1[jt] = t1

            # ---- S-other: one packed gather load + masked adds ---------
            g2a = sb.tile([128, JT * DOUT], bf16, tag="g2a", bufs=1)
            nc.sync.dma_start(
                out=g2a[:].rearrange("p (t n) -> p t n", n=DOUT),
                in_=g2_all[:, :].rearrange("(t p) n -> p t n", p=128))
            for k in range(IT):
                stmp = sb.tile([128, DOUT], bf16, tag="stmp", bufs=3)
                nc.vector.scalar_tensor_tensor(stmp[:], g2a[:, ts(k, DOUT)],
                                               mlo[:, 0:1], p1[IT + k][:],
                                               op0=AOP.mult, op1=AOP.add)
                nc.vector.scalar_tensor_tensor(p1[IT + k][:],
                                               g2a[:, ts(IT + k, DOUT)],
                                               mhi[:, 0:1], stmp[:],
                                               op0=AOP.mult, op1=AOP.add)

            # ---- phase 3: H = (r x X)@Wk0 + alphaT^T S ----------------
            with tc.tile_pool(name="psC", bufs=1, space="PSUM") as psC:
                hps = [psC.tile([128, DOUT], f32, tag=f"h{i}", bufs=1,
                               name=f"h{i}") for i in range(IT)]
                for it in range(IT):
                    for d in range(DT):
                        nc.tensor.matmul(hps[it][:],
                                         lhsT=xts[d][:, ts(it, 128)],
                                         rhs=wk["wk0"][:, ts(d, DOUT)],
                                         start=(d == 0), stop=False)
                for jt in range(IT):
                    for it in range(IT):
                        nc.tensor.matmul(hps[it][:],
                                         lhsT=atS(jt, it * 128, 128),
                                         rhs=p1[jt][:],
                                         start=False, stop=False)
                for it_half in (range(0, IT // 2), range(IT // 2, IT)):
                    for it in it_half:
                        for jt in range(IT, JT):
                            nc.tensor.matmul(hps[it][:],
                                             lhsT=atS(jt, it * 128, 128),
                                             rhs=p1[jt][:],
                                             start=False,
                                             stop=(jt == JT - 1))
                        o = sb.tile([128, DOUT], f32, tag="osb", bufs=3)
                        nc.scalar.activation(o[:], hps[it][:], AF.Relu,
                                             scale=rr_col[:, it:it + 1])
                        nc.sync.dma_start(out=OUT[ts(it, 128), :], in_=o[:])

    nc.compile()
    return nc


def _prep_inputs(X, A, Wv, bv, aw, ab, Wk, bk):
    import ml_dtypes

    bf16 = ml_dtypes.bfloat16
    X = np.asarray(X, np.float32)
    A = np.asarray(A, np.float32)
    Wv = np.asarray(Wv, np.float32)
    bv = np.asarray(bv, np.float32)
    aw = np.asarray(aw, np.float32)
    ab = np.asarray(ab, np.float32)
    Wk = np.asarray(Wk, np.float32)
    bk = np.asarray(bk, np.float32)

    w1 = Wv @ aw[:DOUT, 0]
    c1 = float(bv @ aw[:DOUT, 0])
    w2 = Wv @ aw[DOUT:, 0]
    c2 = float(bv @ aw[DOUT:, 0]) + float(ab[0])
    bks = bk.sum(axis=0).astype(np.float32)
    bks128 = np.ascontiguousarray(np.broadcast_to(bks[None, :], (128, DOUT)),
                                  dtype=np.float32)

    wk_b = [np.ascontiguousarray(Wk[k]).astype(bf16) for k in range(3)]
    in_maps = []
    for c in range(NCORES):
        b, hf = c // 2, c % 2
        own = slice(hf * HALF, (hf + 1) * HALF)
        oth = slice((1 - hf) * HALF, (2 - hf) * HALF)
        perm = np.r_[np.arange(own.start, own.stop),
                     np.arange(oth.start, oth.stop)]
        Xb = X[b]
        sj = (Xb @ w1 + c1).astype(np.float32)
        si = (Xb @ w2 + c2).astype(np.float32)
        sib128 = np.ascontiguousarray(
            np.broadcast_to(si[own][None, :], (128, HALF)), dtype=np.float32)
        in_maps.append({
            "AT": np.ascontiguousarray(A[b][own, :].T[perm, :]).astype(bf16),
            "XT": np.ascontiguousarray(Xb.T[:, perm]).astype(bf16),
            "WK0": wk_b[0], "WK1": wk_b[1], "WK2": wk_b[2],
            "SJT": np.ascontiguousarray(sj[perm].reshape(JT, 128).T,
                                        np.float32),
            "SIB": sib128,
            "BKS": bks128,
            "MLO": np.full((1, 1), 1.0 if hf == 1 else 0.0, np.float32),
            "MHI": np.full((1, 1), 1.0 if hf == 0 else 0.0, np.float32),
        })
    return in_maps


LAST_RESULTS = None


def kernel(X, A, Wv, bv, aw, ab, Wk, bk):
    from concourse.bass_utils import run_bass_kernel_spmd

    if "nc" not in _CACHE:
        _CACHE["nc"] = _build()
    nc = _CACHE["nc"]

    in_maps = _prep_inputs(X, A, Wv, bv, aw, ab, Wk, bk)
    res = run_bass_kernel_spmd(nc, in_maps, core_ids=list(range(NCORES)))
    global LAST_RESULTS
    LAST_RESULTS = res

    out = np.empty((B, N, DOUT), np.float32)
    for c in range(NCORES):
        b, hf = c // 2, c % 2
        out[b, hf * HALF:(hf + 1) * HALF, :] = res.results[c]["out"]
    return out


# revision 25
# speedup vs baseline: 1.0310x; 1.0310x over previous
"""Trainium2 Bass kernel for nn_ADCLayer (GAT-style message passing).

Math (reference reduction):
  sj = X @ (Wv @ aw[:d]) + bv.aw[:d]          (per-column score, j axis)
  si = X @ (Wv @ aw[d:]) + bv.aw[d:] + ab     (per-row score, i axis)
  alpha = A * exp(leaky_relu(si[i] + sj[j]))  (unnormalized transition)
  T = alpha / rowsum(alpha)
  H = X@Wk0 + (T X)@Wk1 + (T^2 X)@Wk2 + sum_k bk[k]   (last ref hop is dead code)
  out = relu(H)

Device algebra (per core, partition=j layout, zero big transposes, both
hops run on RAW alphaT so nothing waits for normalization):
  alphaT[j, i] = A^T[j, i] * exp(lrelu(si[i] + sj[j]))      (bf16)
  r via ones-stationary matmul -> (1, I); recip_col via 8 tiny PE
  transposes + exact reciprocal; r128 row-broadcast via ones outer-product.
  P2 = X@Wk2 ; G2 = recip_col x (alphaT^T P2) + bks -> pairwise AllGather.
  P1 = X@Wk1 ; S-own = P1 + G2-own ; S-other = P1 + masked partner half.
  H_psum = (r x X)@Wk0 + alphaT^T S ; out = relu(recip_col * H_psum).

Sharding: 8 cores = 4 batches x 2 row-halves; j axis permuted per core
(own half first) so own j-tiles have uniform local indices.

DMA strategy: few big multi-dim DMAs (issue op costs ~0.65us of engine
time each), priority emission order, all on the sync queue.
"""

import numpy as np

B, N, DIN, DOUT = 4, 2048, 512, 512
HALF = N // 2          # rows per core
NCORES = 8
JT = N // 128          # 16 j tiles
IT = HALF // 128       # 8 i tiles (also own j tiles)
DT = DIN // 128        # 4 d tiles

_CACHE = {}


def _build():
    import concourse.bacc as bacc
    import concourse.tile as tile
    import concourse.mybir as mybir
    from concourse.bass import ts

    f32 = mybir.dt.float32
    bf16 = mybir.dt.bfloat16
    AOP = mybir.AluOpType
    AF = mybir.ActivationFunctionType

    nc = bacc.Bacc("TRN2", target_bir_lowering=False, debug=False,
                   num_devices=NCORES)

    AT = nc.declare_dram_parameter("AT", [N, HALF], bf16, isOutput=False)
    XT = nc.declare_dram_parameter("XT", [DIN, N], bf16, isOutput=False)
    WK0 = nc.declare_dram_parameter("WK0", [DIN, DOUT], bf16, isOutput=False)
    WK1 = nc.declare_dram_parameter("WK1", [DIN, DOUT], bf16, isOutput=False)
    WK2 = nc.declare_dram_parameter("WK2", [DIN, DOUT], bf16, isOutput=False)
    SJT = nc.declare_dram_parameter("SJT", [128, JT], f32, isOutput=False)
    SIB = nc.declare_dram_parameter("SIB", [128, HALF], f32, isOutput=False)
    BKS = nc.declare_dram_parameter("BKS", [128, DOUT], f32, isOutput=False)
    MLO = nc.declare_dram_parameter("MLO", [1, 1], f32, isOutput=False)
    MHI = nc.declare_dram_parameter("MHI", [1, 1], f32, isOutput=False)
    OUT = nc.declare_dram_parameter("out", [HALF, DOUT], f32, isOutput=True)

    g2_in = nc.dram_tensor("g2_in", [HALF, DOUT], bf16)
    g2_all = nc.dram_tensor("g2_all", [N, DOUT], bf16)

    with tile.TileContext(nc) as tc:
        with tc.tile_pool(name="sb", bufs=1) as sb:
            # ---- input DMAs: individual tiles, priority order, sync ----
            sjt = sb.tile([128, JT], f32, tag="sjt", bufs=1)
            nc.sync.dma_start(out=sjt[:], in_=SJT[:, :])
            sib = sb.tile([128, HALF], f32, tag="sib", bufs=1)
            nc.sync.dma_start(out=sib[:], in_=SIB[:, :])
            at = []
            for jt in range(2):
                t = sb.tile([128, HALF], bf16, tag=f"at{jt}", bufs=1)
                nc.sync.dma_start(out=t[:], in_=AT[ts(jt, 128), :])
                at.append(t)
            xt = []
            for d in range(DT):
                t = sb.tile([128, N], bf16, tag=f"xt{d}", bufs=1)
                nc.sync.dma_start(out=t[:], in_=XT[ts(d, 128), :])
                xt.append(t)
            wk = {}
            for nm, src_ in (("wk2", WK2),):
                wk[nm] = []
                for d in range(DT):
                    t = sb.tile([128, DOUT], bf16, tag=f"{nm}_{d}", bufs=1)
                    nc.sync.dma_start(out=t[:], in_=src_[ts(d, 128), :])
                    wk[nm].append(t)
            for jt in range(2, JT):
                t = sb.tile([128, HALF], bf16, tag=f"at{jt}", bufs=1)
                nc.sync.dma_start(out=t[:], in_=AT[ts(jt, 128), :])
                at.append(t)
            for nm, src_ in (("wk1", WK1), ("wk0", WK0)):
                wk[nm] = []
                for d in range(DT):
                    t = sb.tile([128, DOUT], bf16, tag=f"{nm}_{d}", bufs=1)
                    nc.sync.dma_start(out=t[:], in_=src_[ts(d, 128), :])
                    wk[nm].append(t)
            bks = sb.tile([128, DOUT], f32, tag="bks", bufs=1)
            nc.sync.dma_start(out=bks[:], in_=BKS[:, :])
            mlo = sb.tile([128, 1], f32, tag="mlo", bufs=1)
            nc.sync.dma_start(out=mlo[:], in_=MLO[:, :].to_broadcast((128, 1)))
            mhi = sb.tile([128, 1], f32, tag="mhi", bufs=1)
            nc.sync.dma_start(out=mhi[:], in_=MHI[:, :].to_broadcast((128, 1)))
            ones = sb.tile([128, 1], bf16, tag="ones", bufs=1)
            nc.vector.memset(ones[:], 1.0)
            ones1f = sb.tile([1, 1], f32, tag="ones1f", bufs=1)
            nc.vector.memset(ones1f[:], 1.0)
            ones128f = sb.tile([1, 128], f32, tag="ones128f", bufs=1)
            nc.vector.memset(ones128f[:], 1.0)

            def atS(jt, off, size):
                return at[jt][:, off: off + size]

            p1 = [None] * JT
            with tc.tile_pool(name="psAll", bufs=1, space="PSUM") as psA:
                # ---- phase 1: elementwise alphaT (raw) + r + P2 --------
                r_ps = psA.tile([1, HALF], f32, tag="r", bufs=1)
                for jt in range(JT):
                    z = sb.tile([128, HALF], f32, tag="z", bufs=3)
                    nc.scalar.activation(z[:], sib[:], AF.Identity,
                                         bias=sjt[:, jt:jt + 1], scale=1.0)
                    nc.vector.scalar_tensor_tensor(z[:], z[:], 0.2, z[:],
                                                   op0=AOP.mult, op1=AOP.max)
                    e = sb.tile([128, HALF], bf16, tag="e", bufs=3)
                    nc.scalar.activation(e[:], z[:], AF.Exp)
                    nc.vector.tensor_mul(atS(jt, 0, HALF), atS(jt, 0, HALF),
                                         e[:])
                    for h in range(2):
                        nc.tensor.matmul(r_ps[:, ts(h, 512)], lhsT=ones[:],
                                         rhs=atS(jt, h * 512, 512),
                                         start=(jt == 0), stop=(jt == JT - 1))

                p2 = []
                for jt in range(JT):
                    pp2 = psA.tile([128, DOUT], f32, tag="mm", bufs=2)
                    for d in range(DT):
                        nc.tensor.matmul(pp2[:], lhsT=xt[d][:, ts(jt, 128)],
                                         rhs=wk["wk2"][d][:],
                                         start=(d == 0), stop=(d == DT - 1))
                    t2 = sb.tile([128, DOUT], bf16, tag=f"p2_{jt}", bufs=1)
                    nc.vector.tensor_copy(t2[:], pp2[:])
                    p2.append(t2)

                # ---- hop A on RAW alphaT: two 4-bank passes ------------
                g2sb = [None] * IT
                ua = [None] * IT
                for half in range(2):
                    for i in range(4):
                        it = half * 4 + i
                        ua[it] = psA.tile([128, DOUT], f32, tag=f"ua{i}",
                                          bufs=1, name=f"ua{half}_{i}")
                    for jt in range(JT):
                        for i in range(4):
                            it = half * 4 + i
                            nc.tensor.matmul(ua[it][:],
                                             lhsT=atS(jt, it * 128, 128),
                                             rhs=p2[jt][:],
                                             start=(jt == 0),
                                             stop=(jt == JT - 1))

                    if half == 0:
                        # r -> recip_col (8 tiny PE transposes + recip)
                        r_sb = sb.tile([1, HALF], f32, tag="rsb", bufs=1)
                        nc.vector.tensor_copy(r_sb[:], r_ps[:])
                        rt = psA.tile([128, IT], f32, tag="r", bufs=1,
                                      name="rt")
                        for c in range(IT):
                            nc.tensor.matmul(rt[:, c:c + 1],
                                             lhsT=r_sb[0:1, ts(c, 128)],
                                             rhs=ones1f[:],
                                             is_transpose=True,
                                             start=True, stop=True)
                        rr_col = sb.tile([128, IT], f32, tag="rrc", bufs=1)
                        nc.vector.reciprocal(rr_col[:], rt[:])

                    # G2 = recip_col * UA + bks, straight to the gather
                    for i in range(4):
                        it = half * 4 + i
                        g2t = sb.tile([128, DOUT], bf16, tag=f"g2o{it}",
                                      bufs=1, name=f"g2o{it}")
                        nc.vector.scalar_tensor_tensor(
                            g2t[:], ua[it][:], rr_col[:, it:it + 1], bks[:],
                            op0=AOP.mult, op1=AOP.add)
                        g2sb[it] = g2t
                        nc.sync.dma_start(out=g2_in[ts(it, 128), :],
                                          in_=g2t[:])

                nc.gpsimd.collective_compute(
                    "AllGather", AOP.bypass,
                    ins=[g2_in.ap().opt()],
                    outs=[g2_all.ap().opt()],
                    replica_groups=[[0, 1], [2, 3], [4, 5], [6, 7]],
                )

                # r128 row-broadcast via ones outer product (into psum)
                r128 = psA.tile([128, HALF], f32, tag="r", bufs=1,
                                name="r128")
                for h in range(2):
                    nc.tensor.matmul(r128[:, ts(h, 512)], lhsT=ones128f[:],
                                     rhs=r_sb[0:1, ts(h, 512)],
                                     start=True, stop=True)
                xts = []
                for d in range(DT):
                    t = sb.tile([128, HALF], bf16, tag=f"xts{d}", bufs=1)
                    nc.vector.tensor_mul(t[:], xt[d][:, 0:HALF], r128[:])
                    xts.append(t)

                # ---- cover window: P1 (+S-own fused) -------------------
                for jt in range(JT):
                    pp1 = psA.tile([128, DOUT], f32, tag="mm", bufs=2)
                    for d in range(DT):
                        nc.tensor.matmul(pp1[:], lhsT=xt[d][:, ts(jt, 128)],
                                         rhs=wk["wk1"][d][:],
                                         start=(d == 0), stop=(d == DT - 1))
                    t1 = sb.tile([128, DOUT], bf16, tag=f"p1_{jt}", bufs=1)
                    if jt < IT:
                        nc.vector.scalar_tensor_tensor(
                            t1[:], pp1[:], 1.0, g2sb[jt][:],
                            op0=AOP.mult, op1=AOP.add)
                    else:
                        nc.vector.tensor_copy(t1[:], pp1[:])
                    p1[jt] = t1

            # ---- S-other: partner half via masked add -----------------
            for k in range(IT):
                glo = sb.tile([128, DOUT], bf16, tag=f"glo{k}", bufs=1,
                              name=f"glo{k}")
                nc.sync.dma_start(out=glo[:], in_=g2_all[ts(k, 128), :])
                ghi = sb.tile([128, DOUT], bf16, tag=f"ghi{k}", bufs=1,
                              name=f"ghi{k}")
                nc.sync.dma_start(out=ghi[:], in_=g2_all[ts(IT + k, 128), :])
                stmp = sb.tile([128, DOUT], bf16, tag="stmp", bufs=3)
                nc.vector.scalar_tensor_tensor(stmp[:], glo[:], mlo[:, 0:1],
                                               p1[IT + k][:],
                                               op0=AOP.mult, op1=AOP.add)
                nc.vector.scalar_tensor_tensor(p1[IT + k][:], ghi[:],
                                               mhi[:, 0:1], stmp[:],
                                               op0=AOP.mult, op1=AOP.add)

            # ---- phase 3: H = (r x X)@Wk0 + alphaT^T S ----------------
            with tc.tile_pool(name="psC", bufs=1, space="PSUM") as psC:
                hps = [psC.tile([128, DOUT], f32, tag=f"h{i}", bufs=1,
                               name=f"h{i}") for i in range(IT)]
                for it in range(IT):
                    for d in range(DT):
                        nc.tensor.matmul(hps[it][:],
                                         lhsT=xts[d][:, ts(it, 128)],
                                         rhs=wk["wk0"][d][:],
                                         start=(d == 0), stop=False)
                for jt in range(IT):
                    for it in range(IT):
                        nc.tensor.matmul(hps[it][:],
                                         lhsT=atS(jt, it * 128, 128),
                                         rhs=p1[jt][:],
                                         start=False, stop=False)
                for it_half in (range(0, IT // 2), range(IT // 2, IT)):
                    for it in it_half:
                        for jt in range(IT, JT):
                            nc.tensor.matmul(hps[it][:],
                                             lhsT=atS(jt, it * 128, 128),
                                             rhs=p1[jt][:],
                                             start=False,
                                             stop=(jt == JT - 1))
                        o = sb.tile([128, DOUT], f32, tag="osb", bufs=3)
                        nc.scalar.activation(o[:], hps[it][:], AF.Relu,
                                             scale=rr_col[:, it:it + 1])
                        nc.sync.dma_start(out=OUT[ts(it, 128), :], in_=o[:])

    nc.compile()
    return nc


def _prep_inputs(X, A, Wv, bv, aw, ab, Wk, bk):
    import ml_dtypes

    bf16 = ml_dtypes.bfloat16
    X = np.asarray(X, np.float32)
    A = np.asarray(A, np.float32)
    Wv = np.asarray(Wv, np.float32)
    bv = np.asarray(bv, np.float32)
    aw = np.asarray(aw, np.float32)
    ab = np.asarray(ab, np.float32)
    Wk = np.asarray(Wk, np.float32)
    bk = np.asarray(bk, np.float32)

    w1 = Wv @ aw[:DOUT, 0]
    c1 = float(bv @ aw[:DOUT, 0])
    w2 = Wv @ aw[DOUT:, 0]
    c2 = float(bv @ aw[DOUT:, 0]) + float(ab[0])
    bks = bk.sum(axis=0).astype(np.float32)
    bks128 = np.ascontiguousarray(np.broadcast_to(bks[None, :], (128, DOUT)),
                                  dtype=np.float32)

    wk_b = [np.ascontiguousarray(Wk[k]).astype(bf16) for k in range(3)]
    in_maps = []
    for c in range(NCORES):
        b, hf = c // 2, c % 2
        own = slice(hf * HALF, (hf + 1) * HALF)
        oth = slice((1 - hf) * HALF, (2 - hf) * HALF)
        perm = np.r_[np.arange(own.start, own.stop),
                     np.arange(oth.start, oth.stop)]
        Xb = X[b]
        sj = (Xb @ w1 + c1).astype(np.float32)
        si = (Xb @ w2 + c2).astype(np.float32)
        sib128 = np.ascontiguousarray(
            np.broadcast_to(si[own][None, :], (128, HALF)), dtype=np.float32)
        in_maps.append({
            "AT": np.ascontiguousarray(A[b][own, :].T[perm, :]).astype(bf16),
            "XT": np.ascontiguousarray(Xb.T[:, perm]).astype(bf16),
            "WK0": wk_b[0], "WK1": wk_b[1], "WK2": wk_b[2],
            "SJT": np.ascontiguousarray(sj[perm].reshape(JT, 128).T,
                                        np.float32),
            "SIB": sib128,
            "BKS": bks128,
            "MLO": np.full((1, 1), 1.0 if hf == 1 else 0.0, np.float32),
            "MHI": np.full((1, 1), 1.0 if hf == 0 else 0.0, np.float32),
        })
    return in_maps


LAST_RESULTS = None


def kernel(X, A, Wv, bv, aw, ab, Wk, bk):
    from concourse.bass_utils import run_bass_kernel_spmd

    if "nc" not in _CACHE:
        _CACHE["nc"] = _build()
    nc = _CACHE["nc"]

    in_maps = _prep_inputs(X, A, Wv, bv, aw, ab, Wk, bk)
    res = run_bass_kernel_spmd(nc, in_maps, core_ids=list(range(NCORES)))
    global LAST_RESULTS
    LAST_RESULTS = res

    out = np.empty((B, N, DOUT), np.float32)
    for c in range(NCORES):
        b, hf = c // 2, c % 2
        out[b, hf * HALF:(hf + 1) * HALF, :] = res.results[c]["out"]
    return out


# revision 26
# speedup vs baseline: 1.0318x; 1.0007x over previous
"""Trainium2 Bass kernel for nn_ADCLayer (GAT-style message passing).

Math (reference reduction):
  sj = X @ (Wv @ aw[:d]) + bv.aw[:d]          (per-column score, j axis)
  si = X @ (Wv @ aw[d:]) + bv.aw[d:] + ab     (per-row score, i axis)
  alpha = A * exp(leaky_relu(si[i] + sj[j]))  (unnormalized transition)
  T = alpha / rowsum(alpha)
  H = X@Wk0 + (T X)@Wk1 + (T^2 X)@Wk2 + sum_k bk[k]   (last ref hop is dead code)
  out = relu(H)

Device algebra (per core, partition=j layout, zero big transposes, both
hops run on RAW alphaT so nothing waits for normalization):
  alphaT[j, i] = A^T[j, i] * exp(lrelu(si[i] + sj[j]))      (bf16)
  r via ones-stationary matmul -> (1, I); recip_col via 8 tiny PE
  transposes + exact reciprocal; r128 row-broadcast via ones outer-product.
  P2 = X@Wk2 ; G2 = recip_col x (alphaT^T P2) + bks -> pairwise AllGather.
  P1 = X@Wk1 ; S-own = P1 + G2-own ; S-other = P1 + masked partner half.
  H_psum = (r x X)@Wk0 + alphaT^T S ; out = relu(recip_col * H_psum).

Sharding: 8 cores = 4 batches x 2 row-halves; j axis permuted per core
(own half first) so own j-tiles have uniform local indices.

DMA strategy: few big multi-dim DMAs (issue op costs ~0.65us of engine
time each), priority emission order, all on the sync queue.
"""

import numpy as np

B, N, DIN, DOUT = 4, 2048, 512, 512
HALF = N // 2          # rows per core
NCORES = 8
JT = N // 128          # 16 j tiles
IT = HALF // 128       # 8 i tiles (also own j tiles)
DT = DIN // 128        # 4 d tiles

_CACHE = {}


def _build():
    import concourse.bacc as bacc
    import concourse.tile as tile
    import concourse.mybir as mybir
    from concourse.bass import ts
    from concourse.tile_rust import add_dep_helper

    f32 = mybir.dt.float32
    bf16 = mybir.dt.bfloat16
    AOP = mybir.AluOpType
    AF = mybir.ActivationFunctionType

    nc = bacc.Bacc("TRN2", target_bir_lowering=False, debug=False,
                   num_devices=NCORES)

    AT = nc.declare_dram_parameter("AT", [N, HALF], bf16, isOutput=False)
    XT = nc.declare_dram_parameter("XT", [DIN, N], bf16, isOutput=False)
    WK0 = nc.declare_dram_parameter("WK0", [DIN, DOUT], bf16, isOutput=False)
    WK1 = nc.declare_dram_parameter("WK1", [DIN, DOUT], bf16, isOutput=False)
    WK2 = nc.declare_dram_parameter("WK2", [DIN, DOUT], bf16, isOutput=False)
    SJT = nc.declare_dram_parameter("SJT", [128, JT], f32, isOutput=False)
    SIB = nc.declare_dram_parameter("SIB", [128, HALF], f32, isOutput=False)
    BKS = nc.declare_dram_parameter("BKS", [128, DOUT], f32, isOutput=False)
    MLO = nc.declare_dram_parameter("MLO", [1, 1], f32, isOutput=False)
    MHI = nc.declare_dram_parameter("MHI", [1, 1], f32, isOutput=False)
    OUT = nc.declare_dram_parameter("out", [HALF, DOUT], f32, isOutput=True)

    g2_in = nc.dram_tensor("g2_in", [HALF, DOUT], bf16)
    g2_all = nc.dram_tensor("g2_all", [N, DOUT], bf16)

    with tile.TileContext(nc) as tc:
        with tc.tile_pool(name="sb", bufs=1) as sb:
            # ---- input DMAs: fast class unchained, rest chained --------
            fast = []
            sjt = sb.tile([128, JT], f32, tag="sjt", bufs=1)
            fast.append(nc.sync.dma_start(out=sjt[:], in_=SJT[:, :]))
            sib = sb.tile([128, HALF], f32, tag="sib", bufs=1)
            fast.append(nc.sync.dma_start(out=sib[:], in_=SIB[:, :]))
            at = []
            for jt in range(2):
                t = sb.tile([128, HALF], bf16, tag=f"at{jt}", bufs=1)
                fast.append(nc.sync.dma_start(out=t[:], in_=AT[ts(jt, 128), :]))
                at.append(t)
            xt = []
            for d in range(DT):
                t = sb.tile([128, N], bf16, tag=f"xt{d}", bufs=1)
                fast.append(nc.sync.dma_start(out=t[:], in_=XT[ts(d, 128), :]))
                xt.append(t)
            wk = {}
            for nm, src_ in (("wk2", WK2),):
                wk[nm] = []
                for d in range(DT):
                    t = sb.tile([128, DOUT], bf16, tag=f"{nm}_{d}", bufs=1)
                    fast.append(nc.sync.dma_start(out=t[:], in_=src_[ts(d, 128), :]))
                    wk[nm].append(t)

            def chain(dma):
                for p in fast:
                    add_dep_helper(dma.ins, p.ins, reason="dma priority")
                return dma

            for jt in range(2, JT):
                t = sb.tile([128, HALF], bf16, tag=f"at{jt}", bufs=1)
                chain(nc.sync.dma_start(out=t[:], in_=AT[ts(jt, 128), :]))
                at.append(t)
            for nm, src_ in (("wk1", WK1), ("wk0", WK0)):
                wk[nm] = []
                for d in range(DT):
                    t = sb.tile([128, DOUT], bf16, tag=f"{nm}_{d}", bufs=1)
                    chain(nc.sync.dma_start(out=t[:], in_=src_[ts(d, 128), :]))
                    wk[nm].append(t)
            bks = sb.tile([128, DOUT], f32, tag="bks", bufs=1)
            chain(nc.sync.dma_start(out=bks[:], in_=BKS[:, :]))
            mlo = sb.tile([128, 1], f32, tag="mlo", bufs=1)
            chain(nc.sync.dma_start(out=mlo[:],
                                    in_=MLO[:, :].to_broadcast((128, 1))))
            mhi = sb.tile([128, 1], f32, tag="mhi", bufs=1)
            chain(nc.sync.dma_start(out=mhi[:],
                                    in_=MHI[:, :].to_broadcast((128, 1))))
            ones = sb.tile([128, 1], bf16, tag="ones", bufs=1)
            nc.vector.memset(ones[:], 1.0)
            ones1f = sb.tile([1, 1], f32, tag="ones1f", bufs=1)
            nc.vector.memset(ones1f[:], 1.0)
            ones128f = sb.tile([1, 128], f32, tag="ones128f", bufs=1)
            nc.vector.memset(ones128f[:], 1.0)

            def atS(jt, off, size):
                return at[jt][:, off: off + size]

            p1 = [None] * JT
            with tc.tile_pool(name="psAll", bufs=1, space="PSUM") as psA:
                # ---- phase 1: elementwise alphaT (raw) + r + P2 --------
                r_ps = psA.tile([1, HALF], f32, tag="r", bufs=1)
                for jt in range(JT):
                    z = sb.tile([128, HALF], f32, tag="z", bufs=3)
                    nc.scalar.activation(z[:], sib[:], AF.Identity,
                                         bias=sjt[:, jt:jt + 1], scale=1.0)
                    nc.vector.scalar_tensor_tensor(z[:], z[:], 0.2, z[:],
                                                   op0=AOP.mult, op1=AOP.max)
                    e = sb.tile([128, HALF], bf16, tag="e", bufs=3)
                    nc.scalar.activation(e[:], z[:], AF.Exp)
                    nc.vector.tensor_mul(atS(jt, 0, HALF), atS(jt, 0, HALF),
                                         e[:])
                    for h in range(2):
                        nc.tensor.matmul(r_ps[:, ts(h, 512)], lhsT=ones[:],
                                         rhs=atS(jt, h * 512, 512),
                                         start=(jt == 0), stop=(jt == JT - 1))

                p2 = []
                for jt in range(JT):
                    pp2 = psA.tile([128, DOUT], f32, tag="mm", bufs=2)
                    for d in range(DT):
                        nc.tensor.matmul(pp2[:], lhsT=xt[d][:, ts(jt, 128)],
                                         rhs=wk["wk2"][d][:],
                                         start=(d == 0), stop=(d == DT - 1))
                    t2 = sb.tile([128, DOUT], bf16, tag=f"p2_{jt}", bufs=1)
                    nc.vector.tensor_copy(t2[:], pp2[:])
                    p2.append(t2)

                # ---- hop A on RAW alphaT: two 4-bank passes ------------
                g2sb = [None] * IT
                ua = [None] * IT
                for half in range(2):
                    for i in range(4):
                        it = half * 4 + i
                        ua[it] = psA.tile([128, DOUT], f32, tag=f"ua{i}",
                                          bufs=1, name=f"ua{half}_{i}")
                    for jt in range(JT):
                        for i in range(4):
                            it = half * 4 + i
                            nc.tensor.matmul(ua[it][:],
                                             lhsT=atS(jt, it * 128, 128),
                                             rhs=p2[jt][:],
                                             start=(jt == 0),
                                             stop=(jt == JT - 1))

                    if half == 0:
                        # r -> recip_col (8 tiny PE transposes + recip)
                        r_sb = sb.tile([1, HALF], f32, tag="rsb", bufs=1)
                        nc.vector.tensor_copy(r_sb[:], r_ps[:])
                        rt = psA.tile([128, IT], f32, tag="r", bufs=1,
                                      name="rt")
                        for c in range(IT):
                            nc.tensor.matmul(rt[:, c:c + 1],
                                             lhsT=r_sb[0:1, ts(c, 128)],
                                             rhs=ones1f[:],
                                             is_transpose=True,
                                             start=True, stop=True)
                        rr_col = sb.tile([128, IT], f32, tag="rrc", bufs=1)
                        nc.vector.reciprocal(rr_col[:], rt[:])

                    # G2 = recip_col * UA + bks, straight to the gather
                    for i in range(4):
                        it = half * 4 + i
                        g2t = sb.tile([128, DOUT], bf16, tag=f"g2o{it}",
                                      bufs=1, name=f"g2o{it}")
                        nc.vector.scalar_tensor_tensor(
                            g2t[:], ua[it][:], rr_col[:, it:it + 1], bks[:],
                            op0=AOP.mult, op1=AOP.add)
                        g2sb[it] = g2t
                        nc.sync.dma_start(out=g2_in[ts(it, 128), :],
                                          in_=g2t[:])

                nc.gpsimd.collective_compute(
                    "AllGather", AOP.bypass,
                    ins=[g2_in.ap().opt()],
                    outs=[g2_all.ap().opt()],
                    replica_groups=[[0, 1], [2, 3], [4, 5], [6, 7]],
                )

                # r128 row-broadcast via ones outer product (into psum)
                r128 = psA.tile([128, HALF], f32, tag="r", bufs=1,
                                name="r128")
                for h in range(2):
                    nc.tensor.matmul(r128[:, ts(h, 512)], lhsT=ones128f[:],
                                     rhs=r_sb[0:1, ts(h, 512)],
                                     start=True, stop=True)
                xts = []
                for d in range(DT):
                    t = sb.tile([128, HALF], bf16, tag=f"xts{d}", bufs=1)
                    nc.vector.tensor_mul(t[:], xt[d][:, 0:HALF], r128[:])
                    xts.append(t)

                # ---- cover window: P1 (+S-own fused) -------------------
                for jt in range(JT):
                    pp1 = psA.tile([128, DOUT], f32, tag="mm", bufs=2)
                    for d in range(DT):
                        nc.tensor.matmul(pp1[:], lhsT=xt[d][:, ts(jt, 128)],
                                         rhs=wk["wk1"][d][:],
                                         start=(d == 0), stop=(d == DT - 1))
                    t1 = sb.tile([128, DOUT], bf16, tag=f"p1_{jt}", bufs=1)
                    if jt < IT:
                        nc.vector.scalar_tensor_tensor(
                            t1[:], pp1[:], 1.0, g2sb[jt][:],
                            op0=AOP.mult, op1=AOP.add)
                    else:
                        nc.vector.tensor_copy(t1[:], pp1[:])
                    p1[jt] = t1

            # ---- S-other: partner half via masked add -----------------
            for k in range(IT):
                glo = sb.tile([128, DOUT], bf16, tag=f"glo{k}", bufs=1,
                              name=f"glo{k}")
                nc.sync.dma_start(out=glo[:], in_=g2_all[ts(k, 128), :])
                ghi = sb.tile([128, DOUT], bf16, tag=f"ghi{k}", bufs=1,
                              name=f"ghi{k}")
                nc.sync.dma_start(out=ghi[:], in_=g2_all[ts(IT + k, 128), :])
                stmp = sb.tile([128, DOUT], bf16, tag="stmp", bufs=3)
                nc.vector.scalar_tensor_tensor(stmp[:], glo[:], mlo[:, 0:1],
                                               p1[IT + k][:],
                                               op0=AOP.mult, op1=AOP.add)
                nc.vector.scalar_tensor_tensor(p1[IT + k][:], ghi[:],
                                               mhi[:, 0:1], stmp[:],
                                               op0=AOP.mult, op1=AOP.add)

            # ---- phase 3: H = (r x X)@Wk0 + alphaT^T S ----------------
            with tc.tile_pool(name="psC", bufs=1, space="PSUM") as psC:
                hps = [psC.tile([128, DOUT], f32, tag=f"h{i}", bufs=1,
                               name=f"h{i}") for i in range(IT)]
                for it in range(IT):
                    for d in range(DT):
                        nc.tensor.matmul(hps[it][:],
                                         lhsT=xts[d][:, ts(it, 128)],
                                         rhs=wk["wk0"][d][:],
                                         start=(d == 0), stop=False)
                for jt in range(IT):
                    for it in range(IT):
                        nc.tensor.matmul(hps[it][:],
                                         lhsT=atS(jt, it * 128, 128),
                                         rhs=p1[jt][:],
                                         start=False, stop=False)
                for it_half in (range(0, IT // 2), range(IT // 2, IT)):
                    for it in it_half:
                        for jt in range(IT, JT):
                            nc.tensor.matmul(hps[it][:],
                                             lhsT=atS(jt, it * 128, 128),
                                             rhs=p1[jt][:],
                                             start=False,
                                             stop=(jt == JT - 1))
                        o = sb.tile([128, DOUT], f32, tag="osb", bufs=3)
                        nc.scalar.activation(o[:], hps[it][:], AF.Relu,
                                             scale=rr_col[:, it:it + 1])
                        nc.sync.dma_start(out=OUT[ts(it, 128), :], in_=o[:])

    nc.compile()
    return nc


def _prep_inputs(X, A, Wv, bv, aw, ab, Wk, bk):
    import ml_dtypes

    bf16 = ml_dtypes.bfloat16
    X = np.asarray(X, np.float32)
    A = np.asarray(A, np.float32)
    Wv = np.asarray(Wv, np.float32)
    bv = np.asarray(bv, np.float32)
    aw = np.asarray(aw, np.float32)
    ab = np.asarray(ab, np.float32)
    Wk = np.asarray(Wk, np.float32)
    bk = np.asarray(bk, np.float32)

    w1 = Wv @ aw[:DOUT, 0]
    c1 = float(bv @ aw[:DOUT, 0])
    w2 = Wv @ aw[DOUT:, 0]
    c2 = float(bv @ aw[DOUT:, 0]) + float(ab[0])
    bks = bk.sum(axis=0).astype(np.float32)
    bks128 = np.ascontiguousarray(np.broadcast_to(bks[None, :], (128, DOUT)),
                                  dtype=np.float32)

    wk_b = [np.ascontiguousarray(Wk[k]).astype(bf16) for k in range(3)]
    in_maps = []
    for c in range(NCORES):
        b, hf = c // 2, c % 2
        own = slice(hf * HALF, (hf + 1) * HALF)
        oth = slice((1 - hf) * HALF, (2 - hf) * HALF)
        perm = np.r_[np.arange(own.start, own.stop),
                     np.arange(oth.start, oth.stop)]
        Xb = X[b]
        sj = (Xb @ w1 + c1).astype(np.float32)
        si = (Xb @ w2 + c2).astype(np.float32)
        sib128 = np.ascontiguousarray(
            np.broadcast_to(si[own][None, :], (128, HALF)), dtype=np.float32)
        in_maps.append({
            "AT": np.ascontiguousarray(A[b][own, :].T[perm, :]).astype(bf16),
            "XT": np.ascontiguousarray(Xb.T[:, perm]).astype(bf16),
            "WK0": wk_b[0], "WK1": wk_b[1], "WK2": wk_b[2],
            "SJT": np.ascontiguousarray(sj[perm].reshape(JT, 128).T,
                                        np.float32),
            "SIB": sib128,
            "BKS": bks128,
            "MLO": np.full((1, 1), 1.0 if hf == 1 else 0.0, np.float32),
            "MHI": np.full((1, 1), 1.0 if hf == 0 else 0.0, np.float32),
        })
    return in_maps


LAST_RESULTS = None


def kernel(X, A, Wv, bv, aw, ab, Wk, bk):
    from concourse.bass_utils import run_bass_kernel_spmd

    if "nc" not in _CACHE:
        _CACHE["nc"] = _build()
    nc = _CACHE["nc"]

    in_maps = _prep_inputs(X, A, Wv, bv, aw, ab, Wk, bk)
    res = run_bass_kernel_spmd(nc, in_maps, core_ids=list(range(NCORES)))
    global LAST_RESULTS
    LAST_RESULTS = res

    out = np.empty((B, N, DOUT), np.float32)
    for c in range(NCORES):
        b, hf = c // 2, c % 2
        out[b, hf * HALF:(hf + 1) * HALF, :] = res.results[c]["out"]
    return out
